# revision 1
# baseline (speedup 1.0000x reference)
"""GNN message-passing (chained scatter-mean) Bass kernel for TRN2.

Design (per direction; fwd = mean over in-edges at dst, rev = at src):
- Nodes sharded across 8 cores by destination id; within each core nodes are
  degree-sorted (desc) so 128-node tiles have homogeneous degree.
- Global gather-table layout = concat of per-core padded slices [NPC_pad].
  An AllGather rebuilds the full table in each core's DRAM every round.
- dma_gather (custom Q7 ucode) fetches h[src] rows (64 f32 = 256B elements).
  Indices are int16, so the table is processed in 4 chunks of Ntab/4 rows.
  Chunk membership of a src = its table position // CH; a greedy balancing
  pass assigns table positions so each dst's in-edges spread evenly over
  chunks (minimizes per-(tile,chunk) slot padding).
- Per (chunk, tile) block: K[t,c] slot columns, one gather column group per
  slot; DVE strided reduce over K sums each dst's slots; first chunk writes,
  later chunks accumulate; one tensor_scalar mul applies 1/deg.
"""

import numpy as np

import concourse.bacc as bacc
import concourse.bass as bass
import concourse.mybir as mybir
import concourse.tile as tile

P = 128
NCHUNK = 4
MAXCOL = 32  # max slot-columns per dma_gather instruction (4096 idxs)
F32 = mybir.dt.float32
I16 = mybir.dt.int16


# ---------------------------------------------------------------- host side


def choose_owner(src, dst, N, n_cores, n_sweeps=2, seed=0, n_batches=8):
    """Assign each node an owner core (NPC per core) such that every
    aggregation center's gathered-neighbor chunk counts are balanced
    (chunk = owner-core pair).  Vectorized batched refinement."""
    C = NCHUNK
    rng = np.random.default_rng(seed)
    chunk = rng.permutation(np.arange(N) % C)

    deg_f = np.bincount(dst, minlength=N)
    deg_r = np.bincount(src, minlength=N)
    tgt_f = -(-deg_f // C)
    tgt_r = -(-deg_r // C)

    batches = np.array_split(rng.permutation(N), n_batches)
    for _ in range(n_sweeps):
        for batch in batches:
            nb = len(batch)
            inb = np.zeros(N, dtype=bool)
            inb[batch] = True
            m_f = inb[src]  # edges whose gathered node (src) is in batch
            m_r = inb[dst]
            cnt_f = np.zeros((N, C), dtype=np.int32)
            np.add.at(cnt_f, (dst, chunk[src]), 1)
            np.add.at(cnt_f, (dst[m_f], chunk[src[m_f]]), -1)
            cnt_r = np.zeros((N, C), dtype=np.int32)
            np.add.at(cnt_r, (src, chunk[dst]), 1)
            np.add.at(cnt_r, (src[m_r], chunk[dst[m_r]]), -1)
            full_f = (cnt_f >= tgt_f[:, None]).astype(np.int32)
            full_r = (cnt_r >= tgt_r[:, None]).astype(np.int32)
            bidx = np.empty(N, dtype=np.int64)
            bidx[batch] = np.arange(nb)
            cost = np.zeros((nb, C), dtype=np.int64)
            np.add.at(cost, bidx[src[m_f]], full_f[dst[m_f]] * 1000 + cnt_f[dst[m_f]])
            np.add.at(cost, bidx[dst[m_r]], full_r[src[m_r]] * 1000 + cnt_r[src[m_r]])
            sizes = np.bincount(chunk, minlength=C)
            sizes[chunk[batch]] -= 1  # approximate: remove batch mass
            cost += (sizes * (4 * 1000 // N if N else 0))[None, :]
            chunk[batch] = np.argmin(cost, axis=1)
    return _owner_from_chunks(chunk, N, n_cores, C, rng)


def _owner_from_chunks(chunk, N, n_cores, C, rng):
    per = n_cores // C
    NPC = N // n_cores
    want = NPC * per
    # move surplus nodes (random choice) to deficit chunks
    sizes = np.bincount(chunk, minlength=C)
    surplus = []
    for c in range(C):
        if sizes[c] > want:
            idxs = np.where(chunk == c)[0]
            mv = rng.choice(idxs, size=sizes[c] - want, replace=False)
            surplus.append(mv)
    if surplus:
        surplus = np.concatenate(surplus)
        pos = 0
        for c in range(C):
            if sizes[c] < want:
                need = want - sizes[c]
                chunk[surplus[pos : pos + need]] = c
                pos += need
    owner = np.empty(N, dtype=np.int64)
    for c in range(C):
        nodes = np.where(chunk == c)[0]
        for j in range(per):
            owner[nodes[j * NPC : (j + 1) * NPC]] = c * per + j
    return owner


def _direction_prep(s, d, N, n_cores, owner):
    """Preprocess one direction: edges (s -> d), sharded by owner[d]."""
    E = s.shape[0]
    NPC = N // n_cores
    assert NPC * n_cores == N
    assert NPC % P != 0, "need pad rows to serve as guaranteed zero gather rows"

    deg = np.bincount(d, minlength=N).astype(np.int64)
    inv = np.where(deg > 0, 1.0 / np.maximum(deg, 1), 0.0).astype(np.float32)

    # chunk of a node as gather source = its owner core pair
    node_chunk = owner // (n_cores // NCHUNK)
    # per-(center, chunk) in-edge counts — K[t, c] maxes over these, so sort
    # nodes within each core by max-chunk-count (minimizes slot padding)
    cnt = np.zeros((N, NCHUNK), dtype=np.int32)
    np.add.at(cnt, (d, node_chunk[s]), 1)
    key = cnt.max(axis=1).astype(np.int64) * 100000 + deg

    perm = np.empty(N, dtype=np.int64)
    for c in range(n_cores):
        nodes = np.where(owner == c)[0]
        assert len(nodes) == NPC, f"core {c} owns {len(nodes)} != {NPC}"
        order = np.argsort(-key[nodes], kind="stable")
        perm[c * NPC : (c + 1) * NPC] = nodes[order]
    gid = np.empty(N, dtype=np.int64)  # node -> rank in permuted layout
    gid[perm] = np.arange(N)

    T = -(-NPC // P)
    NPC_pad = T * P
    Ntab = n_cores * NPC_pad
    CH = Ntab // NCHUNK
    assert CH <= 32767, "chunk must fit int16 indexing"
    # table position: core's slice is padded
    gid2 = (gid // NPC) * NPC_pad + (gid % NPC)
    ZERO_LID = NPC  # first pad row of the chunk's first core slice
    assert np.array_equal(gid2 // CH, node_chunk)
    node_lid = gid2 % CH

    # per-tile-per-chunk K
    cnt_pad = np.zeros((n_cores, NPC_pad, NCHUNK), dtype=np.int64)
    cnt_pad[:, :NPC] = cnt[perm].reshape(n_cores, NPC, NCHUNK)
    K = cnt_pad.reshape(n_cores, T, P, NCHUNK).max(axis=(0, 2))  # [T, NCHUNK]
    K[:, 0] = np.maximum(K[:, 0], 1)  # chunk 0 always writes each tile

    # column layout: chunk-major, then tile
    colbase = np.zeros((NCHUNK, T), dtype=np.int64)
    pos = 0
    for c in range(NCHUNK):
        for t in range(T):
            colbase[c, t] = pos
            pos += K[t, c]
    S_tot = pos

    img = np.full((n_cores, P, S_tot), ZERO_LID, dtype=np.int16)

    # slot index k within (dst, chunk)
    ckey = node_chunk[s]
    order = np.argsort(d * NCHUNK + ckey, kind="stable")
    ds, ss, cs = d[order], s[order], ckey[order]
    gkey = ds * NCHUNK + cs
    seg_start = np.searchsorted(gkey, np.arange(N * NCHUNK))
    k = np.arange(E) - seg_start[gkey]
    core = gid[ds] // NPC
    r = gid[ds] - core * NPC
    t = r // P
    p = r % P
    img[core, p, colbase[cs, t] + k] = node_lid[ss].astype(np.int16)

    inv_pad = np.zeros((n_cores, NPC_pad), dtype=np.float32)
    inv_pad[:, :NPC] = inv[perm].reshape(n_cores, NPC)
    inv_img = inv_pad.reshape(n_cores, T, P).transpose(0, 2, 1).copy()  # [nc, P, T]

    # ---- instruction schedule: pack whole (c,t) blocks, <= MAXCOL columns
    instrs = []  # (chunk, col_lo, [(t0, ntiles, K)], width)
    for c in range(NCHUNK):
        cur_tiles = []
        cur_lo = None
        cur_w = 0
        for t in range(T):
            w = int(K[t, c])
            if w == 0:
                continue
            assert w <= MAXCOL, f"K[{t},{c}]={w} exceeds MAXCOL"
            if cur_w + w > MAXCOL:
                instrs.append((c, cur_lo, cur_tiles, cur_w))
                cur_tiles, cur_lo, cur_w = [], None, 0
            if cur_lo is None:
                cur_lo = int(colbase[c, t])
            # group consecutive equal-K tiles for batched reduce
            if cur_tiles and cur_tiles[-1][0] + cur_tiles[-1][1] == t and cur_tiles[
                -1
            ][2] == w:
                cur_tiles[-1] = (cur_tiles[-1][0], cur_tiles[-1][1] + 1, w)
            else:
                cur_tiles.append((t, 1, w))
            cur_w += w
        if cur_tiles:
            instrs.append((c, cur_lo, cur_tiles, cur_w))

    # ---- wrapped int16 index image, one block of width*8 cols per instr
    blocks = []
    for c, lo, tiles, w in instrs:
        blk = img[:, :, lo : lo + w]  # [n_cores, P, w]
        flat = blk.transpose(0, 2, 1).reshape(n_cores, w * P)  # i = col*128 + p
        wrapped = flat.reshape(n_cores, w * 8, 16).transpose(0, 2, 1)  # [nc, 16, w*8]
        blocks.append(np.tile(wrapped, (1, 8, 1)))  # [nc, 128, w*8]
    off_img = np.concatenate(blocks, axis=2)  # [n_cores, P, 8*sum(w)]

    return dict(
        perm=perm,
        Ks=K,
        S=S_tot,
        T=T,
        NPC=NPC,
        NPC_pad=NPC_pad,
        CH=CH,
        instrs=instrs,
        off_img=off_img,
        inv_img=inv_img,
    )


def preprocess(topic, edge_index, n_cores, balance_sweeps=0):
    """Build per-core input maps + assembly info for both directions."""
    N, D = topic.shape
    src, dst = edge_index[0].astype(np.int64), edge_index[1].astype(np.int64)

    if balance_sweeps > 0:
        owner = choose_owner(src, dst, N, n_cores, n_sweeps=balance_sweeps)
    else:
        owner = np.arange(N) // (N // n_cores)
    fwd = _direction_prep(src, dst, N, n_cores, owner)
    rev = _direction_prep(dst, src, N, n_cores, owner)

    NPC, NPC_pad = fwd["NPC"], fwd["NPC_pad"]
    Ntab = n_cores * NPC_pad

    h0_fwd = np.zeros((Ntab, D), dtype=np.float32)
    h0_fwd.reshape(n_cores, NPC_pad, D)[:, :NPC] = topic[fwd["perm"]].reshape(
        n_cores, NPC, D
    )
    h0_rev = np.zeros((Ntab, D), dtype=np.float32)
    h0_rev.reshape(n_cores, NPC_pad, D)[:, :NPC] = topic[rev["perm"]].reshape(
        n_cores, NPC, D
    )

    in_maps = []
    for c in range(n_cores):
        in_maps.append(
            {
                "h0_fwd": h0_fwd,
                "h0_rev": h0_rev,
                "off_fwd": fwd["off_img"][c],
                "off_rev": rev["off_img"][c],
                "inv_fwd": fwd["inv_img"][c],
                "inv_rev": rev["inv_img"][c],
            }
        )
    return fwd, rev, in_maps


def assemble(fwd, rev, outs, N, D, R):
    """outs: per-core 'out' arrays [2R, NPC_pad, D] -> [2R, N, D]."""
    n_cores = len(outs)
    NPC = fwd["NPC"]
    full = np.empty((2 * R, N, D), dtype=np.float32)
    cat_f = np.concatenate([o[:R, :NPC] for o in outs], axis=1)
    cat_r = np.concatenate([o[R:, :NPC] for o in outs], axis=1)
    for q in range(R):
        full[q][fwd["perm"]] = cat_f[q]
        full[R + q][rev["perm"]] = cat_r[q]
    return full


# -------------------------------------------------------------- device side


def build_nc(n_cores, D, R, N, fwd, rev, gather_bufs=4):
    T = fwd["T"]
    assert rev["T"] == T
    NPC = fwd["NPC"]
    NPC_pad = fwd["NPC_pad"]
    Ntab = n_cores * NPC_pad
    CH = fwd["CH"]

    nc = bacc.Bacc(
        "TRN2",
        target_bir_lowering=False,
        debug=False,
        num_devices=n_cores,
        num_swdge_queues=4,
    )

    h0_f = nc.dram_tensor("h0_fwd", [Ntab, D], F32, kind="ExternalInput").ap()
    h0_r = nc.dram_tensor("h0_rev", [Ntab, D], F32, kind="ExternalInput").ap()
    offs = {}
    for dirn, dd in (("f", fwd), ("r", rev)):
        w8 = dd["off_img"].shape[2]
        offs[dirn] = nc.dram_tensor(
            f"off_{'fwd' if dirn == 'f' else 'rev'}", [P, w8], I16, kind="ExternalInput"
        ).ap()
    inv_f = nc.dram_tensor("inv_fwd", [P, T], F32, kind="ExternalInput").ap()
    inv_r = nc.dram_tensor("inv_rev", [P, T], F32, kind="ExternalInput").ap()
    out = nc.dram_tensor("out", [2 * R, NPC_pad, D], F32, kind="ExternalOutput").ap()

    groups = [list(range(n_cores))]

    with tile.TileContext(nc) as tc:
        with (
            tc.tile_pool(name="persist", bufs=1) as pp,
            tc.tile_pool(name="gather", bufs=gather_bufs) as gp,
            tc.tile_pool(name="tmp", bufs=2) as tp,
            tc.tile_pool(name="slices", bufs=1) as sp,
            tc.tile_pool(name="dram", bufs=1, space="DRAM") as dp,
        ):
            off_sb = {}
            for dirn, dd in (("f", fwd), ("r", rev)):
                w8 = dd["off_img"].shape[2]
                off_sb[dirn] = pp.tile([P, w8], I16, name=f"off_sb_{dirn}")
                nc.sync.dma_start(off_sb[dirn][:], offs[dirn][:])
            inv_sb = {"f": pp.tile([P, T], F32, name="inv_sb_f")}
            nc.sync.dma_start(inv_sb["f"][:], inv_f[:])
            inv_sb["r"] = pp.tile([P, T], F32, name="inv_sb_r")
            nc.sync.dma_start(inv_sb["r"][:], inv_r[:])

            htabs = {}
            for dirn in ("f", "r"):
                for rnd in range(1, R):
                    htabs[dirn, rnd] = dp.tile(
                        [Ntab, D], F32, addr_space="Shared", name=f"htab_{dirn}{rnd}"
                    )
            slice_d = {
                "f": dp.tile([NPC_pad, D], F32, name="slice_df"),
                "r": dp.tile([NPC_pad, D], F32, name="slice_dr"),
            }

            qn = 0
            for rnd in range(R):
                for dirn, dd, h0 in (("f", fwd, h0_f), ("r", rev, h0_r)):
                    src_tab = h0 if rnd == 0 else htabs[dirn, rnd][:]
                    slice_sb = sp.tile(
                        [P, T * D], F32, tag=f"slice_{dirn}", name=f"sl_{dirn}{rnd}"
                    )
                    col8 = 0  # column offset into wrapped off image (x8)
                    for j, (c, lo, tiles, w) in enumerate(dd["instrs"]):
                        g = gp.tile(
                            [P, MAXCOL * D], F32, tag="g", name=f"g{dirn}{rnd}_{j}"
                        )
                        nc.gpsimd.dma_gather(
                            out_ap=g[:, : w * D].rearrange("p (g f) -> p g f", f=D),
                            in_ap=src_tab[c * CH : (c + 1) * CH, :],
                            idxs_ap=off_sb[dirn][:, col8 : col8 + w * 8],
                            num_idxs=w * P,
                            num_idxs_reg=w * P,
                            elem_size=D,
                            single_packet=False,
                            queue_num=qn % 4,
                        )
                        qn += 1
                        col8 += w * 8
                        # batched reduces over equal-K tile groups
                        ofs = 0
                        for t0, ntiles, kk in tiles:
                            red_in = g[:, ofs * D : (ofs + ntiles * kk) * D].rearrange(
                                "p (g k f) -> p g f k", k=kk, f=D
                            )
                            sl_view = slice_sb[:, t0 * D : (t0 + ntiles) * D]
                            if c == 0:
                                nc.vector.tensor_reduce(
                                    out=sl_view,
                                    in_=red_in,
                                    axis=mybir.AxisListType.X,
                                    op=mybir.AluOpType.add,
                                )
                            else:
                                tmp = tp.tile(
                                    [P, MAXCOL * D],
                                    F32,
                                    tag="tmp",
                                    name=f"tm{dirn}{rnd}_{j}_{t0}",
                                )
                                nc.vector.tensor_reduce(
                                    out=tmp[:, : ntiles * D],
                                    in_=red_in,
                                    axis=mybir.AxisListType.X,
                                    op=mybir.AluOpType.add,
                                )
                                nc.vector.tensor_add(
                                    sl_view, sl_view, tmp[:, : ntiles * D]
                                )
                            ofs += ntiles * kk

                    for t in range(T):
                        nc.vector.tensor_scalar_mul(
                            slice_sb[:, t * D : (t + 1) * D],
                            slice_sb[:, t * D : (t + 1) * D],
                            inv_sb[dirn][:, t : t + 1],
                        )

                    q = rnd if dirn == "f" else R + rnd
                    sb3 = slice_sb[:].rearrange("p (t f) -> p t f", f=D)
                    out_view = out[q].rearrange("(t p) f -> p t f", p=P)
                    nc.sync.dma_start(out_view, sb3)
                    if rnd < R - 1:
                        sl = slice_d[dirn]
                        sl_view = sl[:].rearrange("(t p) f -> p t f", p=P)
                        nc.sync.dma_start(sl_view, sb3)
                        nc.gpsimd.collective_compute(
                            "AllGather",
                            mybir.AluOpType.bypass,
                            replica_groups=groups,
                            ins=[sl[:].opt()],
                            outs=[htabs[dirn, rnd + 1][:].opt()],
                        )

    nc.compile()
    return nc


# ------------------------------------------------------------ numpy oracle


def numpy_reference(topic, edge_index, R):
    N, D = topic.shape
    src, dst = edge_index[0].astype(np.int64), edge_index[1].astype(np.int64)
    outs = []
    for s, d in ((src, dst), (dst, src)):
        deg = np.bincount(d, minlength=N).astype(np.float32)
        inv = np.where(deg > 0, 1.0 / np.maximum(deg, 1), 0.0)[:, None]
        h = topic.astype(np.float32)
        for _ in range(R):
            acc = np.zeros((N, D), dtype=np.float32)
            np.add.at(acc, d, h[s])
            h = acc * inv
            outs.append(h)
    return np.stack(outs)


# ------------------------------------------------------------- entry point

from concourse.bass_utils import run_bass_kernel_spmd  # noqa: E402

_CORES = 8
_R = 4


def kernel(topic_one_hot, edge_index):
    topic = np.asarray(topic_one_hot, dtype=np.float32)
    ei = np.asarray(edge_index)
    N, D = topic.shape
    fwd, rev, in_maps = preprocess(topic, ei, _CORES, balance_sweeps=2)
    nc = build_nc(_CORES, D, _R, N, fwd, rev)
    res = run_bass_kernel_spmd(nc, in_maps, core_ids=list(range(_CORES)))
    outs = [res.results[c]["out"] for c in range(_CORES)]
    return assemble(fwd, rev, outs, N, D, _R)



# revision 2
# speedup vs baseline: 4.8124x; 4.8124x over previous
"""GNN message-passing (chained scatter-mean) Bass kernel for TRN2 — v3.

Layout (per direction; fwd = mean over in-edges at dst, rev = at src):
- Nodes sharded across 8 cores by aggregation-center id; within each core,
  nodes are degree-sorted and 128-node tiles carry a uniform slot count K
  across all 4 table chunks (chunk = pair of cores); tiles are reordered by
  K so equal-K runs form large gather groups.
- Host ships only each core's own feature slice; AllGathers build/refresh
  the full gather table in every core's DRAM each round.
- Per group: one idx DMA + 4 chunk dma_gathers (Q7 ucode, 256B rows) into
  one contiguous buffer laid [chunk][tile][slot]; slot-sum + chunk-sum are
  two strided X-reduces on DVE (one when K==1). One broadcast tensor mul
  applies 1/deg for the whole slice.
"""

import numpy as np

import concourse.bacc as bacc
import concourse.bass as bass
import concourse.mybir as mybir
import concourse.tile as tile

P = 128
NCHUNK = 4
CAP = 40  # max nt*K slot-columns per chunk-gather (SBUF-bounded)
F32 = mybir.dt.float32
I16 = mybir.dt.int16


# ---------------------------------------------------------------- host side


def choose_owner(src, dst, N, n_cores, n_sweeps=2, seed=0, n_batches=8):
    """Assign each node an owner core (NPC per core) such that every
    aggregation center's gathered-neighbor chunk counts are balanced
    (chunk = owner-core pair).  Vectorized batched refinement."""
    C = NCHUNK
    rng = np.random.default_rng(seed)
    chunk = rng.permutation(np.arange(N) % C)

    deg_f = np.bincount(dst, minlength=N)
    deg_r = np.bincount(src, minlength=N)
    tgt_f = -(-deg_f // C)
    tgt_r = -(-deg_r // C)

    batches = np.array_split(rng.permutation(N), n_batches)
    for _ in range(n_sweeps):
        for batch in batches:
            nb = len(batch)
            inb = np.zeros(N, dtype=bool)
            inb[batch] = True
            m_f = inb[src]
            m_r = inb[dst]
            cnt_f = np.zeros((N, C), dtype=np.int32)
            np.add.at(cnt_f, (dst, chunk[src]), 1)
            np.add.at(cnt_f, (dst[m_f], chunk[src[m_f]]), -1)
            cnt_r = np.zeros((N, C), dtype=np.int32)
            np.add.at(cnt_r, (src, chunk[dst]), 1)
            np.add.at(cnt_r, (src[m_r], chunk[dst[m_r]]), -1)
            full_f = (cnt_f >= tgt_f[:, None]).astype(np.int32)
            full_r = (cnt_r >= tgt_r[:, None]).astype(np.int32)
            bidx = np.empty(N, dtype=np.int64)
            bidx[batch] = np.arange(nb)
            cost = np.zeros((nb, C), dtype=np.int64)
            np.add.at(cost, bidx[src[m_f]], full_f[dst[m_f]] * 1000 + cnt_f[dst[m_f]])
            np.add.at(cost, bidx[dst[m_r]], full_r[src[m_r]] * 1000 + cnt_r[src[m_r]])
            sizes = np.bincount(chunk, minlength=C)
            sizes[chunk[batch]] -= 1
            cost += (sizes * (4 * 1000 // N if N else 0))[None, :]
            chunk[batch] = np.argmin(cost, axis=1)
    return _owner_from_chunks(chunk, N, n_cores, C, rng)


def _owner_from_chunks(chunk, N, n_cores, C, rng):
    per = n_cores // C
    NPC = N // n_cores
    want = NPC * per
    sizes = np.bincount(chunk, minlength=C)
    surplus = []
    for c in range(C):
        if sizes[c] > want:
            idxs = np.where(chunk == c)[0]
            mv = rng.choice(idxs, size=sizes[c] - want, replace=False)
            surplus.append(mv)
    if surplus:
        surplus = np.concatenate(surplus)
        pos = 0
        for c in range(C):
            if sizes[c] < want:
                need = want - sizes[c]
                chunk[surplus[pos : pos + need]] = c
                pos += need
    owner = np.empty(N, dtype=np.int64)
    for c in range(C):
        nodes = np.where(chunk == c)[0]
        for j in range(per):
            owner[nodes[j * NPC : (j + 1) * NPC]] = c * per + j
    return owner


def _direction_prep(s, d, N, n_cores, owner):
    """Preprocess one direction: edges (s -> d), sharded by owner[d]."""
    E = s.shape[0]
    NPC = N // n_cores
    assert NPC * n_cores == N
    assert NPC % P != 0, "need pad rows to serve as guaranteed zero gather rows"

    deg = np.bincount(d, minlength=N).astype(np.int64)
    inv = np.where(deg > 0, 1.0 / np.maximum(deg, 1), 0.0).astype(np.float32)

    node_chunk = owner // (n_cores // NCHUNK)
    cnt = np.zeros((N, NCHUNK), dtype=np.int32)
    np.add.at(cnt, (d, node_chunk[s]), 1)
    key = cnt.max(axis=1).astype(np.int64) * 100000 + deg

    perm = np.empty(N, dtype=np.int64)
    for c in range(n_cores):
        nodes = np.where(owner == c)[0]
        assert len(nodes) == NPC, f"core {c} owns {len(nodes)} != {NPC}"
        order = np.argsort(-key[nodes], kind="stable")
        perm[c * NPC : (c + 1) * NPC] = nodes[order]

    T = -(-NPC // P)
    NPC_pad = T * P
    Ntab = n_cores * NPC_pad
    CH = Ntab // NCHUNK
    assert CH <= 32767, "chunk must fit int16 indexing"
    ZERO_LID = NPC  # first pad row of the chunk's first core slice

    # per-tile uniform K (max over cores, partitions, chunks)
    gid = np.empty(N, dtype=np.int64)
    gid[perm] = np.arange(N)
    cnt_pad = np.zeros((n_cores, NPC_pad, NCHUNK), dtype=np.int64)
    cnt_pad[:, :NPC] = cnt[perm].reshape(n_cores, NPC, NCHUNK)
    Kp = cnt_pad.reshape(n_cores, T, P, NCHUNK).max(axis=(0, 2, 3))  # [T]
    Kp = np.maximum(Kp, 1)

    # reorder full tiles by K desc (keep the partial last tile in place) so
    # equal-K runs are contiguous; recompose the per-core permutation
    tile_order = np.append(np.argsort(-Kp[: T - 1], kind="stable"), T - 1)
    newperm = np.empty_like(perm)
    for c in range(n_cores):
        cp = perm[c * NPC : (c + 1) * NPC]
        parts = [cp[P * o : P * (o + 1)] for o in tile_order[:-1]]
        parts.append(cp[P * (T - 1) :])
        newperm[c * NPC : (c + 1) * NPC] = np.concatenate(parts)
    perm = newperm
    Kp = Kp[tile_order]
    gid[perm] = np.arange(N)
    gid2 = (gid // NPC) * NPC_pad + (gid % NPC)
    assert np.array_equal(gid2 // CH, node_chunk)
    node_lid = gid2 % CH

    # groups of consecutive equal-K tiles, nt*K <= CAP
    groups = []  # (t0, nt, K)
    t = 0
    while t < T:
        K = int(Kp[t])
        nt = 1
        while t + nt < T and Kp[t + nt] == K and (nt + 1) * K <= CAP:
            nt += 1
        groups.append((t, nt, K))
        t += nt

    # column layout: per group, chunk-major blocks [c][tile][slot]
    gbase = np.empty(len(groups), dtype=np.int64)
    pos = 0
    for gi, (t0, nt, K) in enumerate(groups):
        gbase[gi] = pos
        pos += 4 * nt * K
    S_tot = pos

    # per-tile lookup: group base, offset within group, group's nt*K
    tile_gb = np.empty(T, dtype=np.int64)
    tile_off = np.empty(T, dtype=np.int64)
    tile_ntk = np.empty(T, dtype=np.int64)
    tile_K = np.empty(T, dtype=np.int64)
    for gi, (t0, nt, K) in enumerate(groups):
        for i in range(nt):
            tile_gb[t0 + i] = gbase[gi]
            tile_off[t0 + i] = i * K
            tile_ntk[t0 + i] = nt * K
            tile_K[t0 + i] = K

    img = np.full((n_cores, P, S_tot), ZERO_LID, dtype=np.int16)

    ckey = node_chunk[s]
    order = np.argsort(d * NCHUNK + ckey, kind="stable")
    ds, ss, cs = d[order], s[order], ckey[order]
    gkey = ds * NCHUNK + cs
    seg_start = np.searchsorted(gkey, np.arange(N * NCHUNK))
    k = np.arange(E) - seg_start[gkey]
    core = gid[ds] // NPC
    r = gid[ds] - core * NPC
    t = r // P
    p = r % P
    assert (k < tile_K[t]).all()
    col = tile_gb[t] + cs * tile_ntk[t] + tile_off[t] + k
    img[core, p, col] = node_lid[ss].astype(np.int16)

    inv_pad = np.zeros((n_cores, NPC_pad), dtype=np.float32)
    inv_pad[:, :NPC] = inv[perm].reshape(n_cores, NPC)
    inv_img = inv_pad.reshape(n_cores, T, P).transpose(0, 2, 1).copy()  # [nc,P,T]

    # wrapped int16 index image: one block of 4*nt*K*8 cols per group
    blocks = []
    for gi, (t0, nt, K) in enumerate(groups):
        w = 4 * nt * K
        blk = img[:, :, gbase[gi] : gbase[gi] + w]  # [n_cores, P, w]
        flat = blk.transpose(0, 2, 1).reshape(n_cores, w * P)  # i = col*128 + p
        wrapped = flat.reshape(n_cores, w * 8, 16).transpose(0, 2, 1)
        blocks.append(np.tile(wrapped, (1, 8, 1)))  # [nc, 128, w*8]
    off_img = np.concatenate(blocks, axis=2)  # [n_cores, P, 8*S_tot]

    return dict(
        perm=perm,
        S=S_tot,
        T=T,
        NPC=NPC,
        NPC_pad=NPC_pad,
        CH=CH,
        groups=groups,
        off_img=off_img,
        inv_img=inv_img,
    )


def preprocess(topic, edge_index, n_cores, balance_sweeps=2):
    """Build per-core input maps + assembly info for both directions."""
    N, D = topic.shape
    src, dst = edge_index[0].astype(np.int64), edge_index[1].astype(np.int64)

    if balance_sweeps > 0:
        owner = choose_owner(src, dst, N, n_cores, n_sweeps=balance_sweeps)
    else:
        owner = np.arange(N) // (N // n_cores)
    fwd = _direction_prep(src, dst, N, n_cores, owner)
    rev = _direction_prep(dst, src, N, n_cores, owner)

    NPC, NPC_pad = fwd["NPC"], fwd["NPC_pad"]

    sl_f = np.zeros((n_cores, NPC_pad, D), dtype=np.float32)
    sl_f[:, :NPC] = topic[fwd["perm"]].reshape(n_cores, NPC, D)
    sl_r = np.zeros((n_cores, NPC_pad, D), dtype=np.float32)
    sl_r[:, :NPC] = topic[rev["perm"]].reshape(n_cores, NPC, D)

    in_maps = []
    for c in range(n_cores):
        in_maps.append(
            {
                "h0s_fwd": sl_f[c],
                "h0s_rev": sl_r[c],
                "off_fwd": fwd["off_img"][c],
                "off_rev": rev["off_img"][c],
                "inv_fwd": fwd["inv_img"][c],
                "inv_rev": rev["inv_img"][c],
            }
        )
    return fwd, rev, in_maps


def assemble(fwd, rev, outs, N, D, R):
    """outs: per-core 'out' arrays [2R, NPC_pad, D] -> [2R, N, D]."""
    NPC = fwd["NPC"]
    full = np.empty((2 * R, N, D), dtype=np.float32)
    cat_f = np.concatenate([o[:R, :NPC] for o in outs], axis=1)
    cat_r = np.concatenate([o[R:, :NPC] for o in outs], axis=1)
    for q in range(R):
        full[q][fwd["perm"]] = cat_f[q]
        full[R + q][rev["perm"]] = cat_r[q]
    return full


# -------------------------------------------------------------- device side


def build_nc(n_cores, D, R, N, fwd, rev, gather_bufs=2):
    T = fwd["T"]
    assert rev["T"] == T
    NPC_pad = fwd["NPC_pad"]
    Ntab = n_cores * NPC_pad
    CH = fwd["CH"]

    nc = bacc.Bacc(
        "TRN2",
        target_bir_lowering=False,
        debug=False,
        num_devices=n_cores,
        num_swdge_queues=4,
    )

    h0s = {
        "f": nc.dram_tensor("h0s_fwd", [NPC_pad, D], F32, kind="ExternalInput").ap(),
        "r": nc.dram_tensor("h0s_rev", [NPC_pad, D], F32, kind="ExternalInput").ap(),
    }
    offs = {}
    for dirn, dd in (("f", fwd), ("r", rev)):
        w8 = dd["off_img"].shape[2]
        offs[dirn] = nc.dram_tensor(
            f"off_{'fwd' if dirn == 'f' else 'rev'}", [P, w8], I16, kind="ExternalInput"
        ).ap()
    invs = {
        "f": nc.dram_tensor("inv_fwd", [P, T], F32, kind="ExternalInput").ap(),
        "r": nc.dram_tensor("inv_rev", [P, T], F32, kind="ExternalInput").ap(),
    }
    out = nc.dram_tensor("out", [2 * R, NPC_pad, D], F32, kind="ExternalOutput").ap()

    groups8 = [list(range(n_cores))]

    with tile.TileContext(nc) as tc:
        with (
            tc.tile_pool(name="persist", bufs=1) as pp,
            tc.tile_pool(name="gather", bufs=gather_bufs) as gp,
            tc.tile_pool(name="idx", bufs=4) as ip,
            tc.tile_pool(name="tmp", bufs=2) as tp,
            tc.tile_pool(name="slices", bufs=1) as sp,
            tc.tile_pool(name="dram", bufs=1, space="DRAM") as dp,
        ):
            inv_sb = {}
            for dirn in ("f", "r"):
                inv_sb[dirn] = pp.tile([P, T], F32, name=f"inv_sb_{dirn}")
                nc.sync.dma_start(inv_sb[dirn][:], invs[dirn][:])

            htabs = {}
            for dirn in ("f", "r"):
                for rnd in range(R):
                    htabs[dirn, rnd] = dp.tile(
                        [Ntab, D], F32, addr_space="Shared", name=f"htab_{dirn}{rnd}"
                    )
            slice_d = {
                "f": dp.tile([NPC_pad, D], F32, name="slice_df"),
                "r": dp.tile([NPC_pad, D], F32, name="slice_dr"),
            }

            # initial tables: AllGather of the host-shipped slices (bounced
            # through SBUF: collectives cannot read IO tensors directly)
            h0_d = {
                "f": dp.tile([NPC_pad, D], F32, name="h0_df"),
                "r": dp.tile([NPC_pad, D], F32, name="h0_dr"),
            }
            for dirn in ("f", "r"):
                stage = sp.tile(
                    [P, T * D], F32, tag=f"slice_{dirn}", name=f"h0sb_{dirn}"
                )
                nc.sync.dma_start(
                    stage[:].rearrange("p (t f) -> p t f", f=D),
                    h0s[dirn].rearrange("(t p) f -> p t f", p=P),
                )
                nc.sync.dma_start(
                    h0_d[dirn][:].rearrange("(t p) f -> p t f", p=P),
                    stage[:].rearrange("p (t f) -> p t f", f=D),
                )
                nc.gpsimd.collective_compute(
                    "AllGather",
                    mybir.AluOpType.bypass,
                    replica_groups=groups8,
                    ins=[h0_d[dirn][:].opt()],
                    outs=[htabs[dirn, 0][:].opt()],
                )

            qn = 0
            for rnd in range(R):
                for dirn, dd in (("f", fwd), ("r", rev)):
                    src_tab = htabs[dirn, rnd][:]
                    slice_sb = sp.tile(
                        [P, T * D], F32, tag=f"slice_{dirn}", name=f"sl_{dirn}{rnd}"
                    )
                    col8 = 0
                    for gi, (t0, nt, K) in enumerate(dd["groups"]):
                        m = nt * K
                        idx_sb = ip.tile([P, 4 * CAP * 8], I16, tag="idx")
                        nc.sync.dma_start(
                            idx_sb[:, : 4 * m * 8],
                            offs[dirn][:, col8 : col8 + 4 * m * 8],
                        )
                        g = gp.tile(
                            [P, 4 * CAP * D], F32, tag="g", name=f"g{dirn}{rnd}_{gi}"
                        )
                        for c in range(NCHUNK):
                            nc.gpsimd.dma_gather(
                                out_ap=g[
                                    :, c * m * D : (c + 1) * m * D
                                ].rearrange("p (m f) -> p m f", f=D),
                                in_ap=src_tab[c * CH : (c + 1) * CH, :],
                                idxs_ap=idx_sb[:, c * m * 8 : (c + 1) * m * 8],
                                num_idxs=m * P,
                                num_idxs_reg=m * P,
                                elem_size=D,
                                single_packet=False,
                                queue_num=qn % 4,
                            )
                            qn += 1
                        col8 += 4 * m * 8
                        sl_view = slice_sb[:, t0 * D : (t0 + nt) * D]
                        if K == 1:
                            nc.vector.tensor_reduce(
                                out=sl_view,
                                in_=g[:, : 4 * m * D].rearrange(
                                    "p (c t f) -> p t f c", c=4, f=D
                                ),
                                axis=mybir.AxisListType.X,
                                op=mybir.AluOpType.add,
                            )
                        else:
                            tmp = tp.tile(
                                [P, 4 * (CAP // 2) * D],
                                F32,
                                tag="tmp",
                                name=f"tm{dirn}{rnd}_{gi}",
                            )
                            nc.vector.tensor_reduce(
                                out=tmp[:, : 4 * nt * D],
                                in_=g[:, : 4 * m * D].rearrange(
                                    "p (q k f) -> p q f k", k=K, f=D
                                ),
                                axis=mybir.AxisListType.X,
                                op=mybir.AluOpType.add,
                            )
                            nc.vector.tensor_reduce(
                                out=sl_view,
                                in_=tmp[:, : 4 * nt * D].rearrange(
                                    "p (c t f) -> p t f c", c=4, f=D
                                ),
                                axis=mybir.AxisListType.X,
                                op=mybir.AluOpType.add,
                            )

                    nc.vector.tensor_mul(
                        slice_sb[:].rearrange("p (t f) -> p t f", f=D),
                        slice_sb[:].rearrange("p (t f) -> p t f", f=D),
                        inv_sb[dirn][:].unsqueeze(2).broadcast_to((P, T, D)),
                    )

                    q = rnd if dirn == "f" else R + rnd
                    sb3 = slice_sb[:].rearrange("p (t f) -> p t f", f=D)
                    out_view = out[q].rearrange("(t p) f -> p t f", p=P)
                    nc.sync.dma_start(out_view, sb3)
                    if rnd < R - 1:
                        sl = slice_d[dirn]
                        sl_view = sl[:].rearrange("(t p) f -> p t f", p=P)
                        nc.sync.dma_start(sl_view, sb3)
                        nc.gpsimd.collective_compute(
                            "AllGather",
                            mybir.AluOpType.bypass,
                            replica_groups=groups8,
                            ins=[sl[:].opt()],
                            outs=[htabs[dirn, rnd + 1][:].opt()],
                        )

    nc.compile()
    return nc


# ------------------------------------------------------------- entry point

from concourse.bass_utils import run_bass_kernel_spmd  # noqa: E402

_CORES = 8
_R = 4


def kernel(topic_one_hot, edge_index):
    topic = np.asarray(topic_one_hot, dtype=np.float32)
    ei = np.asarray(edge_index)
    N, D = topic.shape
    fwd, rev, in_maps = preprocess(topic, ei, _CORES, balance_sweeps=2)
    nc = build_nc(_CORES, D, _R, N, fwd, rev)
    res = run_bass_kernel_spmd(nc, in_maps, core_ids=list(range(_CORES)))
    outs = [res.results[c]["out"] for c in range(_CORES)]
    return assemble(fwd, rev, outs, N, D, _R)
